# revision 9
# baseline (speedup 1.0000x reference)
"""Multi-head attention (B=4, S=2048, D=1024, H=16, DH=64) on 8 trn2 cores.

Sharding: tensor-parallel over heads. Core c owns heads (2c, 2c+1).
Each core computes:
  qkvT = W_shard^T @ x^T (feat-major, x transposed on-chip via PE),
  scoresT[k,q] = kT^T . qT  per head / q-tile,
  attnT = exp(SCALE*scoresT)  (no max subtraction; scores are O(5)),
  out65 = [ones|v]^T @ attnT  (row 64 = softmax denominator, free),
  outT = out65[0:64] * recip(out65[64])  -> headsT (feat-major),
  proj partial = headsT^T @ W_out_shard  -> [8192, 1024] per core.
Host: out = sum(partials) + b_out.

All matmuls run in float32r (tf32-like, 1 cyc/row at N>=256); transposes in
plain fp32 (exact). PSUM accumulate is fp32.
"""

import numpy as np

import concourse.bacc as bacc
import concourse.mybir as mybir
import concourse.tile as tile
from concourse.bass_utils import run_bass_kernel_spmd
from concourse.masks import make_identity

B, S, D, H, DH = 4, 2048, 1024, 16, 64
HPC = 2                      # heads per core
NCORES = 8
F = 3 * HPC * DH             # 384 qkv features per core
SCALE = DH ** -0.5
P = 128
TT = 256                     # token tile for qkv projection
NTT = S // TT                # 8 token tiles per batch
QT = 256                     # q tile for attention
NQT = S // QT                # 8
NKB = S // P                 # 16 k blocks
NDC = D // P                 # 8 contraction chunks
NTB = S // P                 # 16 token blocks for proj

F32 = mybir.dt.float32
F32R = mybir.dt.float32r

DEBUG_TAPS = False           # set True (before _build) to dump intermediates


def _r(ap):
    return ap.bitcast(F32R)


def _build():
    nc = bacc.Bacc("TRN2", debug=False, num_devices=NCORES)

    x_d = nc.dram_tensor("x", [B, S, D], F32, kind="ExternalInput")
    wq_d = nc.dram_tensor("w_qkv_shard", [D, F], F32R, kind="ExternalInput")
    bq_d = nc.dram_tensor("b_qkv_shard", [F], F32, kind="ExternalInput")
    wo0_d = nc.dram_tensor("w_out0", [DH, D], F32R, kind="ExternalInput")
    wo1_d = nc.dram_tensor("w_out1", [DH, D], F32R, kind="ExternalInput")
    out_d = nc.dram_tensor("outp", [B, S, D], F32, kind="ExternalOutput")
    taps = {}
    if DEBUG_TAPS:
        taps["xT"] = nc.dram_tensor("dbg_xT", [P, NDC, TT], F32, kind="ExternalOutput")
        taps["qT"] = nc.dram_tensor("dbg_qT", [P, S], F32, kind="ExternalOutput")
        taps["vT"] = nc.dram_tensor("dbg_vT", [P, S], F32, kind="ExternalOutput")
        taps["v1"] = nc.dram_tensor("dbg_v1", [P, NKB, DH + 1], F32, kind="ExternalOutput")
        taps["attnT"] = nc.dram_tensor("dbg_attnT", [P, NKB, QT], F32, kind="ExternalOutput")
        taps["av"] = nc.dram_tensor("dbg_av", [DH + 1, QT], F32, kind="ExternalOutput")
        taps["headsT"] = nc.dram_tensor("dbg_headsT", [DH, S], F32, kind="ExternalOutput")

    with tile.TileContext(nc) as tc:
        with (
            tc.tile_pool(name="const", bufs=1) as constp,
            tc.tile_pool(name="xp", bufs=2) as xp,
            tc.tile_pool(name="xtp", bufs=2) as xtp,
            tc.tile_pool(name="qkvp", bufs=1) as qkvp,
            tc.tile_pool(name="v1p", bufs=2) as v1p,
            tc.tile_pool(name="attp", bufs=2) as attp,
            tc.tile_pool(name="hp", bufs=2) as hp,
            tc.tile_pool(name="rp", bufs=3) as rp,
            tc.tile_pool(name="outsp", bufs=3) as outsp,
            tc.tile_pool(name="ps_t", bufs=2, space="PSUM") as ps_t,
            tc.tile_pool(name="ps_mm", bufs=2, space="PSUM") as ps_mm,
            tc.tile_pool(name="ps_sc", bufs=2, space="PSUM") as ps_sc,
            tc.tile_pool(name="ps_av", bufs=2, space="PSUM") as ps_av,
        ):
            # ---- constants ----
            wq_sb = constp.tile([P, NDC, F], F32R, tag="wq")
            nc.sync.dma_start(
                out=wq_sb[:], in_=wq_d.ap().rearrange("(c p) f -> p c f", p=P)
            )
            bq_sb = constp.tile([P, 3], F32, tag="bq")
            nc.sync.dma_start(
                out=bq_sb[:], in_=bq_d.ap().rearrange("(j p) -> p j", p=P)
            )
            wo_sb = [
                constp.tile([DH, D], F32R, tag=f"wo{h}", name=f"wo{h}")
                for h in range(HPC)
            ]
            nc.sync.dma_start(out=wo_sb[0][:], in_=wo0_d.ap())
            nc.sync.dma_start(out=wo_sb[1][:], in_=wo1_d.ap())
            ident = constp.tile([P, P], F32, tag="ident")
            make_identity(nc, ident[:])
            ones_c = constp.tile([P, NKB], F32, tag="ones")
            nc.vector.memset(ones_c[:], 1.0)

            for b in range(B):
                # ---- qkv projection for batch b (feat-major output) ----
                qkvT = [
                    qkvp.tile([P, S], F32R if j < 2 else F32, tag=f"qkvT{j}", name=f"qkvT{j}_{b}")
                    for j in range(3)
                ]  # q, k, v ; rows = 2 heads x 64
                for tt in range(NTT):
                    x_t = xp.tile([P, TT // P, D], F32, tag="x")
                    nc.sync.dma_start(
                        out=x_t[:],
                        in_=x_d.ap()[b, tt * TT : (tt + 1) * TT, :].rearrange(
                            "(blk p) d -> p blk d", p=P
                        ),
                    )
                    xT = xtp.tile([P, NDC, TT], F32R, tag="xT")
                    for blk in range(TT // P):
                        for dc4 in range(NDC // 4):
                            tp = ps_t.tile([P, 4, P], F32, tag="pst")
                            for j in range(4):
                                dc = dc4 * 4 + j
                                nc.tensor.transpose(
                                    tp[:, j, :],
                                    x_t[:, blk, dc * P : (dc + 1) * P],
                                    ident[:],
                                )
                            nc.vector.tensor_copy(
                                xT[:, dc4 * 4 : (dc4 + 1) * 4, blk * P : (blk + 1) * P],
                                tp[:],
                            )
                    if DEBUG_TAPS and b == 0 and tt == 0:
                        xT_sb = xp.tile([P, NDC, TT], F32, tag="xTdbg")
                        nc.vector.tensor_copy(xT_sb[:], xT[:])
                        nc.sync.dma_start(out=taps["xT"].ap(), in_=xT_sb[:])
                    for ft in range(3):
                        mm = ps_mm.tile([P, TT], F32, tag="mm")
                        for dc in range(NDC):
                            nc.tensor.matmul(
                                mm[:],
                                wq_sb[:, dc, ft * P : (ft + 1) * P],
                                xT[:, dc, :],
                                start=(dc == 0),
                                stop=(dc == NDC - 1),
                            )
                        nc.vector.tensor_scalar_add(
                            qkvT[ft][:, tt * TT : (tt + 1) * TT],
                            mm[:],
                            bq_sb[:, ft : ft + 1],
                        )
                qT, kT, vT = qkvT
                if DEBUG_TAPS and b == 0:
                    nc.sync.dma_start(out=taps["qT"].ap(), in_=qT[:].bitcast(F32))
                    nc.sync.dma_start(out=taps["vT"].ap(), in_=vT[:])

                # ---- v1 = [v | ones] token-major per head ----
                v1 = []
                for h in range(HPC):
                    v1_h = v1p.tile([P, NKB, DH + 1], F32R, tag="v1", name=f"v1_{b}_{h}")
                    nc.vector.tensor_copy(v1_h[:, :, DH], ones_c[:])
                    for kb8 in range(NKB // 8):
                        tp = ps_t.tile([P, 8, DH], F32, tag="pst")
                        for j in range(8):
                            kb = kb8 * 8 + j
                            nc.tensor.transpose(
                                tp[:, j, :],
                                vT[h * DH : (h + 1) * DH, kb * P : (kb + 1) * P],
                                ident[h * DH : (h + 1) * DH, h * DH : (h + 1) * DH],
                            )
                        nc.vector.tensor_copy(
                            v1_h[:, kb8 * 8 : (kb8 + 1) * 8, 0:DH], tp[:]
                        )
                    if DEBUG_TAPS and b == 0 and h == 0:
                        nc.sync.dma_start(out=taps["v1"].ap(), in_=v1_h[:].bitcast(F32))
                    v1.append(v1_h)

                # ---- attention per head / q-tile ----
                headsT = [
                    hp.tile([DH, S], F32R, tag=f"headsT{h}", name=f"headsT{h}_{b}")
                    for h in range(HPC)
                ]
                for h in range(HPC):
                    hs = slice(h * DH, (h + 1) * DH)
                    for qt in range(NQT):
                        qs = slice(qt * QT, (qt + 1) * QT)
                        attnT = attp.tile([P, NKB, QT], F32R, tag="attnT")
                        for kq in range(NKB // 2):
                            sc = ps_sc.tile([P, 2, QT], F32, tag="sc")
                            for j in range(2):
                                kc = kq * 2 + j
                                nc.tensor.matmul(
                                    sc[:, j, :],
                                    kT[hs, kc * P : (kc + 1) * P],
                                    qT[hs, qs],
                                    start=True,
                                    stop=True,
                                )
                            nc.scalar.activation(
                                attnT[:, kq * 2 : (kq + 1) * 2, :],
                                sc[:],
                                mybir.ActivationFunctionType.Exp,
                                bias=0.0,
                                scale=float(SCALE),
                            )
                        av = ps_av.tile([DH + 1, QT], F32, tag="av")
                        for kc in range(NKB):
                            nc.tensor.matmul(
                                av[:],
                                v1[h][:, kc, :],
                                attnT[:, kc, :],
                                start=(kc == 0),
                                stop=(kc == NKB - 1),
                            )
                        if DEBUG_TAPS and b == 0 and h == 0 and qt == 0:
                            nc.sync.dma_start(out=taps["attnT"].ap(), in_=attnT[:].bitcast(F32))
                        recip = rp.tile([DH + 1, QT], F32, tag="recip")
                        nc.vector.reciprocal(
                            recip[DH : DH + 1, :], av[DH : DH + 1, :]
                        )
                        rb0 = rp.tile([1, QT], F32, tag="rb0")
                        nc.sync.dma_start(out=rb0[:], in_=recip[DH : DH + 1, :])
                        rbc = rp.tile([DH, QT], F32, tag="rbc")
                        nc.gpsimd.partition_broadcast(
                            rbc[:], rb0[0:1, :], channels=DH
                        )
                        if DEBUG_TAPS and b == 0 and h == 0 and qt == 0:
                            av_sb = rp.tile([DH + 1, QT], F32, tag="avsb")
                            nc.vector.tensor_copy(av_sb[:], av[:])
                            nc.sync.dma_start(out=taps["av"].ap(), in_=av_sb[:])
                        nc.vector.tensor_mul(
                            headsT[h][:, qs], av[0:DH, :], rbc[:]
                        )

                if DEBUG_TAPS and b == 0:
                    nc.sync.dma_start(out=taps["headsT"].ap(), in_=headsT[0][:].bitcast(F32))
                # ---- output projection (partial over this core's heads) ----
                for tb in range(NTB):
                    ts = slice(tb * P, (tb + 1) * P)
                    stage = outsp.tile([P, D], F32, tag="stage")
                    for half in range(2):
                        ns = slice(half * 512, (half + 1) * 512)
                        pr = ps_mm.tile([P, 512], F32, tag="mm")
                        for h in range(HPC):
                            nc.tensor.matmul(
                                pr[:],
                                headsT[h][:, ts],
                                wo_sb[h][:, ns],
                                start=(h == 0),
                                stop=(h == HPC - 1),
                            )
                        nc.vector.tensor_copy(stage[:, ns], pr[:])
                    nc.sync.dma_start(out=out_d.ap()[b, ts, :], in_=stage[:])

    nc.compile()
    return nc


_NC_CACHE = {}


def _get_nc():
    if "nc" not in _NC_CACHE:
        _NC_CACHE["nc"] = _build()
    return _NC_CACHE["nc"]


def _shard_inputs(x, w_qkv, b_qkv, w_out):
    in_maps = []
    for c in range(NCORES):
        h0 = c * HPC * DH                      # first head-dim row of this core
        cols = []
        for m in range(3):                     # q, k, v blocks
            cols.append(slice(m * D + h0, m * D + h0 + HPC * DH))
        wq = np.concatenate([w_qkv[:, s] for s in cols], axis=1)
        bq = np.concatenate([b_qkv[s] for s in cols])
        wo = w_out[h0 : h0 + HPC * DH, :]
        in_maps.append(
            {
                "x": np.ascontiguousarray(x, dtype=np.float32),
                "w_qkv_shard": np.ascontiguousarray(wq, dtype=np.float32),
                "b_qkv_shard": np.ascontiguousarray(bq, dtype=np.float32),
                "w_out0": np.ascontiguousarray(wo[:DH], dtype=np.float32),
                "w_out1": np.ascontiguousarray(wo[DH:], dtype=np.float32),
            }
        )
    return in_maps


def kernel(x, w_qkv, b_qkv, w_out, b_out):
    nc = _get_nc()
    in_maps = _shard_inputs(
        np.asarray(x), np.asarray(w_qkv), np.asarray(b_qkv), np.asarray(w_out)
    )
    res = run_bass_kernel_spmd(nc, in_maps, core_ids=list(range(NCORES)))
    acc = np.zeros((B, S, D), dtype=np.float32)
    for m in res.results:
        acc += m["outp"]
    acc += np.asarray(b_out, dtype=np.float32)
    return acc
